# revision 42
# baseline (speedup 1.0000x reference)
"""Trainium2 Bass kernel for nn_EnergyToRateConverter.

Computes Eyring rates  fwd = pref*exp(-(bar - G_from)/RT),
rev = reversible ? pref*exp(-(bar - G_to)/RT) : 0  for B=1M batch rows.

Strategy (pure data parallel over 8 cores, batch split 8 ways):
  * Host transposes inputs into "feature-major" layout X = [state.T;
    barrier.T] of shape (80, B) so that the per-transition gather
    G_from/G_to and the barrier subtraction become one small constant
    matmul W.T @ X with contraction over SBUF partitions:
        W[s, j]    = 1  for s == from_idx[j] (fwd cols) / to_idx[j] (rev)
        W[32+j, j] = -1 (subtract barrier j)
    Output columns are [48 fwd | reversible rev | pad-to-16]; rates for
    non-reversible transitions are never computed — the device output
    buffer is pre-zeroed, so those rows are just never written.
  * 80 and the padded M are multiples of 16, which is what the HWDGE
    descriptor->SDMA-engine split needs to use all 16 engines.
  * X is shipped as an fp16 hi + fp8e4m3 lo pair (3 bytes/element, 25%
    less than f32); the two matmul passes accumulate in PSUM, recovering
    ~5e-4 worst-case relative accuracy at one PE cycle per row each.
  * ScalarE evaluates out = exp(x*inv_rt + ln(pref)) straight from PSUM.
  * Input DMAs ride the SP HWDGE ring, output DMAs the ACT ring, so
    output waits never head-of-line-block input prefetch.
"""

import os

import ml_dtypes
import numpy as np

B = 1048576
N_CORES = 8
BC = B // N_CORES  # 131072 batch rows per core
NS = 32
NT = 48
K = NS + NT  # 80 contraction rows: states then barriers

F_SUPER = 4096  # batch columns per DMA super-tile
F_PSUM = 2048  # batch columns per PSUM tile / ACT op
F_MM = 512  # batch columns per matmul (one PSUM bank)

T = 298.15
K_B = 1.380649e-23
H = 6.62607015e-34
R = 0.008314462618
EYRING_PREFACTOR = K_B * T / H
RT = R * T
INV_RT = float(np.float32(1.0 / RT))  # reference casts 1/RT to f32
LN_PREF = float(np.log(EYRING_PREFACTOR))
LO_SCALE = 64.0

_cached = {}


def _build_program(m_out):
    from concourse import bacc, mybir
    from concourse.tile import TileContext

    nc = bacc.Bacc(
        None, target_bir_lowering=False, debug=False, num_devices=N_CORES
    )
    xh = nc.dram_tensor("x_hi", [K, BC], mybir.dt.float16, kind="ExternalInput")
    xl = nc.dram_tensor("x_lo", [K, BC], mybir.dt.float8e4, kind="ExternalInput")
    wh = nc.dram_tensor("w_hi", [K, m_out], mybir.dt.float16, kind="ExternalInput")
    wl = nc.dram_tensor("w_lo", [K, m_out], mybir.dt.float8e4, kind="ExternalInput")
    y = nc.dram_tensor("y", [m_out, BC], mybir.dt.float32, kind="ExternalOutput")

    exp = mybir.ActivationFunctionType.Exp

    with TileContext(nc) as tc:
        with (
            tc.tile_pool(name="consts", bufs=1) as cpool,
            tc.tile_pool(name="inp", bufs=8) as ipool,
            tc.tile_pool(name="outp", bufs=5) as opool,
            tc.tile_pool(name="psum", bufs=2, space="PSUM") as ppool,
        ):
            wth = cpool.tile([K, m_out], mybir.dt.float16)
            nc.sync.dma_start(wth[:], wh[:])
            wtl = cpool.tile([K, m_out], mybir.dt.float8e4)
            nc.sync.dma_start(wtl[:], wl[:])
            bias_t = cpool.tile([128, 1], mybir.dt.float32)
            nc.vector.memset(bias_t[:], LN_PREF)

            def supertile(c0, width, ip, op, tg):
                hi = ip.tile([K, width], mybir.dt.float16, name=f"hi{tg}", tag=f"hi{tg}")
                nc.sync.dma_start(hi[:], xh[:, c0 : c0 + width])
                lo = ip.tile([K, width], mybir.dt.float8e4, name=f"lo{tg}", tag=f"lo{tg}")
                nc.gpsimd.dma_start(lo[:], xl[:, c0 : c0 + width])
                out = op.tile(
                    [m_out, width], mybir.dt.float32, name=f"out{tg}", tag=f"out{tg}"
                )
                for p in range(width // F_PSUM):
                    ps = ppool.tile([m_out, F_PSUM], mybir.dt.float32, name="ps", tag="ps")
                    for m in range(F_PSUM // F_MM):
                        a = p * F_PSUM + m * F_MM
                        s = slice(m * F_MM, (m + 1) * F_MM)
                        nc.tensor.matmul(
                            ps[:, s], wth[:], hi[:, a : a + F_MM],
                            start=True, stop=False,
                        )
                        nc.tensor.matmul(
                            ps[:, s], wtl[:], lo[:, a : a + F_MM],
                            start=False, stop=True,
                        )
                    po = slice(p * F_PSUM, (p + 1) * F_PSUM)
                    nc.scalar.activation(
                        out[:, po], ps[:],
                        exp, bias=bias_t[:m_out], scale=INV_RT,
                    )
                    nc.scalar.dma_start(
                        y[:, c0 + p * F_PSUM : c0 + (p + 1) * F_PSUM], out[:, po]
                    )

            for t in range(BC // F_SUPER):
                supertile(t * F_SUPER, F_SUPER, ipool, opool, "")
    nc.compile()
    return nc


def _host_prep(state_energies, barrier_energies, from_idx, to_idx, reversible):
    se = np.asarray(state_energies, dtype=np.float32)
    be = np.asarray(barrier_energies, dtype=np.float32)
    fi = np.asarray(from_idx).astype(np.int64)
    ti = np.asarray(to_idx).astype(np.int64)
    rv = np.asarray(reversible).astype(bool)

    x = np.empty((K, B), np.float32)
    x[0:NS] = se.T
    x[NS:] = be.T
    xh = x.astype(np.float16)
    # residual scaled by 64 (folded back via w_lo = w/64) to stay in
    # fp8e4m3's normal range
    xl = ((x - xh.astype(np.float32)) * np.float32(LO_SCALE)).astype(
        ml_dtypes.float8_e4m3
    )

    rev_idx = np.flatnonzero(rv)  # transitions with a reverse rate
    n_rev = len(rev_idx)
    m_out = ((NT + n_rev + 15) // 16) * 16

    w = np.zeros((K, m_out), np.float32)
    cols = np.arange(NT)
    w[fi, cols] = 1.0
    w[NS + cols, cols] = -1.0
    rcols = NT + np.arange(n_rev)
    w[ti[rev_idx], rcols] = 1.0
    w[NS + rev_idx, rcols] = -1.0
    wb_hi = w.astype(np.float16)
    wb_lo = (w / np.float32(LO_SCALE)).astype(ml_dtypes.float8_e4m3)
    return xh, xl, wb_hi, wb_lo, rev_idx, m_out


last_results = None


def kernel(state_energies, barrier_energies, from_idx, to_idx, reversible):
    global last_results
    from concourse.bass_utils import run_bass_kernel_spmd

    xh, xl, wb_hi, wb_lo, rev_idx, m_out = _host_prep(
        state_energies, barrier_energies, from_idx, to_idx, reversible
    )

    if m_out not in _cached:
        _cached[m_out] = _build_program(m_out)
    nc = _cached[m_out]

    in_maps = []
    for c in range(N_CORES):
        sl = slice(c * BC, (c + 1) * BC)
        in_maps.append(
            {
                "x_hi": np.ascontiguousarray(xh[:, sl]),
                "x_lo": np.ascontiguousarray(xl[:, sl]),
                "w_hi": wb_hi,
                "w_lo": wb_lo,
            }
        )

    res = run_bass_kernel_spmd(
        nc,
        in_maps,
        core_ids=list(range(N_CORES)),
        trace=bool(int(os.environ.get("KERNEL_TRACE", "0"))),
    )
    last_results = res

    n_rev = len(rev_idx)
    forward = np.empty((B, NT), np.float32)
    reverse = np.zeros((B, NT), np.float32)
    for c, r in enumerate(res.results):
        yc = r["y"]
        forward[c * BC : (c + 1) * BC] = yc[:NT].T
        reverse[c * BC : (c + 1) * BC, rev_idx] = yc[NT : NT + n_rev].T
    return forward, reverse


# revision 43
# speedup vs baseline: 1.0350x; 1.0350x over previous
"""Trainium2 Bass kernel for nn_EnergyToRateConverter.

Computes Eyring rates  fwd = pref*exp(-(bar - G_from)/RT),
rev = reversible ? pref*exp(-(bar - G_to)/RT) : 0  for B=1M batch rows.

Strategy (pure data parallel over 8 cores, batch split 8 ways):
  * Host transposes inputs into "feature-major" layout X = [state.T;
    barrier.T] of shape (80, B) so that the per-transition gather
    G_from/G_to and the barrier subtraction become one small constant
    matmul W.T @ X with contraction over SBUF partitions:
        W[s, j]    = 1  for s == from_idx[j] (fwd cols) / to_idx[j] (rev)
        W[32+j, j] = -1 (subtract barrier j)
    Output columns are [48 fwd | reversible rev | pad-to-16]; rates for
    non-reversible transitions are never computed — the device output
    buffer is pre-zeroed, so those rows are just never written.
  * 80 and the padded M are multiples of 16, which is what the HWDGE
    descriptor->SDMA-engine split needs to use all 16 engines.
  * X is shipped as an fp16 hi + fp8e4m3 lo pair (3 bytes/element, 25%
    less than f32); the two matmul passes accumulate in PSUM, recovering
    ~5e-4 worst-case relative accuracy at one PE cycle per row each.
  * ScalarE evaluates out = exp(x*inv_rt + ln(pref)) straight from PSUM.
  * Input DMAs ride the SP HWDGE ring, output DMAs the ACT ring, so
    output waits never head-of-line-block input prefetch.
"""

import os

import ml_dtypes
import numpy as np

B = 1048576
N_CORES = 8
BC = B // N_CORES  # 131072 batch rows per core
NS = 32
NT = 48
K = NS + NT  # 80 contraction rows: states then barriers

F_SUPER = 4096  # batch columns per DMA super-tile
F_PSUM = 2048  # batch columns per PSUM tile / ACT op
F_MM = 512  # batch columns per matmul (one PSUM bank)

T = 298.15
K_B = 1.380649e-23
H = 6.62607015e-34
R = 0.008314462618
EYRING_PREFACTOR = K_B * T / H
RT = R * T
INV_RT = float(np.float32(1.0 / RT))  # reference casts 1/RT to f32
LN_PREF = float(np.log(EYRING_PREFACTOR))
LO_SCALE = 64.0

_cached = {}


def _build_program(m_out):
    from concourse import bacc, mybir
    from concourse.tile import TileContext

    nc = bacc.Bacc(
        None, target_bir_lowering=False, debug=False, num_devices=N_CORES
    )
    xh = nc.dram_tensor("x_hi", [K, BC], mybir.dt.float16, kind="ExternalInput")
    xl = nc.dram_tensor("x_lo", [K, BC], mybir.dt.float8e4, kind="ExternalInput")
    wh = nc.dram_tensor("w_hi", [K, m_out], mybir.dt.float16, kind="ExternalInput")
    wl = nc.dram_tensor("w_lo", [K, m_out], mybir.dt.float8e4, kind="ExternalInput")
    y = nc.dram_tensor("y", [m_out, BC], mybir.dt.float32, kind="ExternalOutput")

    exp = mybir.ActivationFunctionType.Exp

    with TileContext(nc) as tc:
        with (
            tc.tile_pool(name="consts", bufs=1) as cpool,
            tc.tile_pool(name="inp", bufs=6) as ipool,
            tc.tile_pool(name="outp", bufs=4) as opool,
            tc.tile_pool(name="psum", bufs=2, space="PSUM") as ppool,
        ):
            wth = cpool.tile([K, m_out], mybir.dt.float16)
            nc.sync.dma_start(wth[:], wh[:])
            wtl = cpool.tile([K, m_out], mybir.dt.float8e4)
            nc.sync.dma_start(wtl[:], wl[:])
            bias_t = cpool.tile([128, 1], mybir.dt.float32)
            nc.vector.memset(bias_t[:], LN_PREF)

            def supertile(c0, width, ip, op, tg):
                hi = ip.tile([K, width], mybir.dt.float16, name=f"hi{tg}", tag=f"hi{tg}")
                nc.sync.dma_start(hi[:], xh[:, c0 : c0 + width])
                lo = ip.tile([K, width], mybir.dt.float8e4, name=f"lo{tg}", tag=f"lo{tg}")
                nc.gpsimd.dma_start(lo[:], xl[:, c0 : c0 + width])
                out = op.tile(
                    [m_out, width], mybir.dt.float32, name=f"out{tg}", tag=f"out{tg}"
                )
                for p in range(width // F_PSUM):
                    ps = ppool.tile([m_out, F_PSUM], mybir.dt.float32, name="ps", tag="ps")
                    for m in range(F_PSUM // F_MM):
                        a = p * F_PSUM + m * F_MM
                        s = slice(m * F_MM, (m + 1) * F_MM)
                        nc.tensor.matmul(
                            ps[:, s], wth[:], hi[:, a : a + F_MM],
                            start=True, stop=False,
                        )
                        nc.tensor.matmul(
                            ps[:, s], wtl[:], lo[:, a : a + F_MM],
                            start=False, stop=True,
                        )
                    po = slice(p * F_PSUM, (p + 1) * F_PSUM)
                    nc.scalar.activation(
                        out[:, po], ps[:],
                        exp, bias=bias_t[:m_out], scale=INV_RT,
                    )
                    nc.scalar.dma_start(
                        y[:, c0 + p * F_PSUM : c0 + (p + 1) * F_PSUM], out[:, po]
                    )

            for t in range(BC // F_SUPER):
                supertile(t * F_SUPER, F_SUPER, ipool, opool, "")
    nc.compile()
    return nc


def _host_prep(state_energies, barrier_energies, from_idx, to_idx, reversible):
    se = np.asarray(state_energies, dtype=np.float32)
    be = np.asarray(barrier_energies, dtype=np.float32)
    fi = np.asarray(from_idx).astype(np.int64)
    ti = np.asarray(to_idx).astype(np.int64)
    rv = np.asarray(reversible).astype(bool)

    x = np.empty((K, B), np.float32)
    x[0:NS] = se.T
    x[NS:] = be.T
    xh = x.astype(np.float16)
    # residual scaled by 64 (folded back via w_lo = w/64) to stay in
    # fp8e4m3's normal range
    xl = ((x - xh.astype(np.float32)) * np.float32(LO_SCALE)).astype(
        ml_dtypes.float8_e4m3
    )

    rev_idx = np.flatnonzero(rv)  # transitions with a reverse rate
    n_rev = len(rev_idx)
    m_out = ((NT + n_rev + 15) // 16) * 16

    w = np.zeros((K, m_out), np.float32)
    cols = np.arange(NT)
    w[fi, cols] = 1.0
    w[NS + cols, cols] = -1.0
    rcols = NT + np.arange(n_rev)
    w[ti[rev_idx], rcols] = 1.0
    w[NS + rev_idx, rcols] = -1.0
    wb_hi = w.astype(np.float16)
    wb_lo = (w / np.float32(LO_SCALE)).astype(ml_dtypes.float8_e4m3)
    return xh, xl, wb_hi, wb_lo, rev_idx, m_out


last_results = None


def kernel(state_energies, barrier_energies, from_idx, to_idx, reversible):
    global last_results
    from concourse.bass_utils import run_bass_kernel_spmd

    xh, xl, wb_hi, wb_lo, rev_idx, m_out = _host_prep(
        state_energies, barrier_energies, from_idx, to_idx, reversible
    )

    if m_out not in _cached:
        _cached[m_out] = _build_program(m_out)
    nc = _cached[m_out]

    in_maps = []
    for c in range(N_CORES):
        sl = slice(c * BC, (c + 1) * BC)
        in_maps.append(
            {
                "x_hi": np.ascontiguousarray(xh[:, sl]),
                "x_lo": np.ascontiguousarray(xl[:, sl]),
                "w_hi": wb_hi,
                "w_lo": wb_lo,
            }
        )

    res = run_bass_kernel_spmd(
        nc,
        in_maps,
        core_ids=list(range(N_CORES)),
        trace=bool(int(os.environ.get("KERNEL_TRACE", "0"))),
    )
    last_results = res

    n_rev = len(rev_idx)
    forward = np.empty((B, NT), np.float32)
    reverse = np.zeros((B, NT), np.float32)
    for c, r in enumerate(res.results):
        yc = r["y"]
        forward[c * BC : (c + 1) * BC] = yc[:NT].T
        reverse[c * BC : (c + 1) * BC, rev_idx] = yc[NT : NT + n_rev].T
    return forward, reverse
